# revision 15
# baseline (speedup 1.0000x reference)
"""Trainium2 kernel for nn_Deep_Root_Net.

Device (8 NeuronCores, pure data parallel, 128 samples/core):
  The 6-layer CNN (3x conv k2 VALID + 3x convT k2 stride1) runs on the PE
  array in 32x32 tiling mode: each sample-layer is a chain of PSUM-
  accumulated shifted-tap matmuls; 16 chains run concurrently across the
  16 PE sub-tiles. L1 is a single matmul per sample (taps folded into
  K=32 via host-side im2col). Biases are folded into the PSUM evacuation
  (ACT Identity with a per-partition bias column). LeakyReLU(0.5) =
  ACT copy, then DVE max(0.5*u, u) (bit-exact vs jnp.where).
  ConvT layers read zero-ring-padded SBUF activations so every tap
  covers the full contiguous output region.

Host: gramian + Root-MUSIC (two LAPACK eigs per sample) mirror the
reference implementation verbatim on CPU jax - the eig deflation order and
stable argsort semantics are LAPACK/jax-specific and chaotically sensitive
(a 1e-7 perturbation of Rx flips root orderings O(1)), so the only sound
implementation runs the identical code path on the numerically stable
part of the pipeline (Rx -> Rz is stable; the CNN on device matches a
float64 reference to ~3e-8).
"""
import os
import sys

import numpy as np

for _p in ("/opt/trn_rl_repo", "/root/.axon_site/_ro/trn_rl_repo"):
    if os.path.isdir(_p) and _p not in sys.path:
        sys.path.append(_p)

import concourse.bacc as bacc
import concourse.mybir as mybir
import concourse.tile as tile
from concourse.bass_utils import run_bass_kernel_spmd

F32 = mybir.dt.float32
AF = mybir.ActivationFunctionType
ALU = mybir.AluOpType

N_CORES = 8
BPC = 128      # samples per core
WAVES = 8      # 16 samples per wave
SLOPE = 0.5
EPS = 1.0
NSEN = 16

# kind, Cin, Cout, Hin, Win, Hout, Wout
LAYERS = [
    ("conv",    8, 16, 32, 16, 31, 15),
    ("conv",   16, 32, 31, 15, 30, 14),
    ("conv",   32, 64, 30, 14, 29, 13),
    ("deconv", 64, 32, 29, 13, 30, 14),
    ("deconv", 32, 16, 30, 14, 31, 15),
    ("deconv", 16,  1, 31, 15, 32, 16),
]

# weights blob column map. Tap blocks are per-32-row-quadrant (replicated
# x4); bias columns are full-height [128, 1]. L2/L6 fold the dx taps into
# K=32 (activations carry a x+1-shifted replica on partitions 16..32).
C1IM = 0                                     # L1 im2col lhsT [32, 32] (M pad)
C2F = [32, 64]                               # L2 dy-folded lhsT [32, 32]
C3T = [96, 160, 224, 288]                    # taps [32,64]
C4T = [[352 + 128 * h + 32 * t for t in range(4)] for h in range(2)]
C5T = [608, 640, 656, 672]                   # taps [32,16] (t0,t3 pad to 32)
C6F = [704, 736]                             # L6 dy-folded lhsT [32, 32] (M pad)
BCOL = 768                                   # 6 bias columns [128, 1]
ACOL = 774                                   # alpha column (0.5) [128, 1]
WCOLS = 800


def _weights_blob(p):
    q = np.zeros((32, WCOLS), np.float32)

    def conv_tap(w, dy, dx):       # torch conv w [Cout, Cin, 2, 2] -> [Cin, Cout]
        return np.ascontiguousarray(w[:, :, dy, dx].T)

    def deconv_tap(w, dy, dx):     # torch convT w [Cin, Cout, 2, 2] -> [Cin, Cout]
        return np.ascontiguousarray(w[:, :, 1 - dy, 1 - dx])

    for t in range(4):
        dy, dx = t // 2, t % 2
        q[8 * t:8 * t + 8, C1IM:C1IM + 16] = conv_tap(p["conv1_w"], dy, dx)
        q[16 * dx:16 * dx + 16, C2F[dy]:C2F[dy] + 32] = \
            conv_tap(p["conv2_w"], dy, dx)
        q[0:32, C3T[t]:C3T[t] + 64] = conv_tap(p["conv3_w"], dy, dx)
        for h in range(2):
            q[0:32, C4T[h][t]:C4T[h][t] + 32] = \
                deconv_tap(p["deconv2_w"], dy, dx)[32 * h:32 * h + 32]
        q[0:32, C5T[t]:C5T[t] + 16] = deconv_tap(p["deconv3_w"], dy, dx)
        q[16 * dx:16 * dx + 16, C6F[dy]:C6F[dy] + 1] = \
            deconv_tap(p["deconv4_w"], dy, dx)
    blob = np.tile(q, (4, 1))

    r = np.arange(128)
    bias = np.zeros((128, 6), np.float32)
    c32, c64 = r % 32, r % 64
    bias[:, 0] = np.where(c32 < 16, p["conv1_b"][np.minimum(c32, 15)], 0)
    bias[:, 1] = p["conv2_b"][c32]
    bias[:, 2] = p["conv3_b"][c64]
    bias[:, 3] = p["deconv2_b"][c32]
    bias[:, 4] = np.where(c32 < 16, p["deconv3_b"][np.minimum(c32, 15)], 0)
    bias[:, 5] = np.where(c32 == 0, p["deconv4_b"][0], 0)
    blob[:, BCOL:BCOL + 6] = bias
    blob[:, ACOL] = SLOPE
    return blob


def _pack_x(x_core):
    # im2col: [128, 8, 32, 16] -> [WAVES, 4(a), 32(t*8+c), 4(b), 465]
    # sample = w*16 + 4a + b; row t*8+c = x[c, dy:dy+31, dx:dx+15].ravel()
    xim = np.empty((BPC, 4, 8, 465), np.float32)
    for t in range(4):
        dy, dx = t // 2, t % 2
        xim[:, t] = x_core[:, :, dy:dy + 31, dx:dx + 15].reshape(BPC, 8, 465)
    xim = xim.reshape(BPC, 32, 465)
    xp = np.empty((WAVES, 4, 32, 4, 465), np.float32)
    for w in range(WAVES):
        for a in range(4):
            for b in range(4):
                xp[w, a, :, b] = xim[w * 16 + 4 * a + b]
    return xp.reshape(WAVES, 128, 4, 465)


def _unpack_y(y):
    # y [WAVES, 4(b), 4(a), 32, 16] -> [128, 32, 16]
    rx = np.empty((BPC, 32, 16), np.float32)
    for w in range(WAVES):
        for b in range(4):
            for a in range(4):
                rx[w * 16 + 4 * a + b] = y[w, b, a]
    return rx


def build(waves=WAVES, prelu=True):
    nc = bacc.Bacc("TRN2", target_bir_lowering=False, debug=False)
    x_d = nc.dram_tensor("x", [waves, 128, 4, 465], F32, kind="ExternalInput")
    w_d = nc.dram_tensor("w", [128, WCOLS], F32, kind="ExternalInput")
    y_d = nc.dram_tensor("y", [waves, 4, 4, 32, 16], F32, kind="ExternalOutput")

    with tile.TileContext(nc) as tc:
        with tc.tile_pool(name="wt", bufs=1) as wt, \
             tc.tile_pool(name="xin", bufs=3) as xin, \
             tc.tile_pool(name="act", bufs=3) as act, \
             tc.tile_pool(name="tmp", bufs=6) as tmp, \
             tc.tile_pool(name="ps", bufs=8, space="PSUM") as ps:

            w_t = wt.tile([128, WCOLS], F32, name="wblob")
            nc.sync.dma_start(out=w_t[:, :], in_=w_d[:, :])

            def bcol(i):
                return w_t[:, BCOL + i:BCOL + i + 1]

            def acol():
                return w_t[:, ACOL:ACOL + 1]

            S = [dict() for _ in range(waves)]

            # persistent zero-ring-padded activation buffers (3-deep ring
            # per layer, indexed w%3): rings are zeroed once here; waves
            # overwrite interiors only, so no per-wave memsets.
            pads = {}
            for key, shape in (("a3A", [128, 4, 31, 15]),
                               ("a3B", [128, 4, 31, 15]),
                               ("a4", [128, 4, 32, 16]),
                               ("a5", [128, 4, 33, 17])):
                tiles = []
                for i in range(3):
                    tl = act.tile(shape, F32, name=f"pad_{key}_{i}",
                                  tag=f"pad_{key}_{i}", bufs=1)
                    nc.gpsimd.memset(tl[:], 0.0)
                    tiles.append(tl)
                pads[key] = tiles

            def mm(out, lhsT, rhs, start, stop, pos):
                # skip_group_check: CoreSim's global zero-region bookkeeping
                # mis-addresses partition offsets encoded in psum AP offsets;
                # the per-tile pending-zero semantics (and HW) handle
                # per-partition accumulation chains correctly.
                nc.tensor.matmul(out=out, lhsT=lhsT, rhs=rhs, start=start,
                                 stop=stop, tile_position=pos,
                                 skip_group_check=True)

            def psum4():
                return [ps.tile([128, 512], F32, name=f"ps{nc.next_id()}",
                                tag="ps") for _ in range(4)]

            def evac_leaky(psum_view, bias_ap, dst, name, dve=False):
                if dve:
                    # DVE path: u = psum + bias, then in-place max(0.5u, u).
                    # Splitting evacs across ACT/DVE halves the per-stage
                    # PSUM-release latency (banks evacuate in parallel).
                    nc.vector.tensor_scalar_add(out=dst[:], in0=psum_view[:],
                                                scalar1=bias_ap)
                    nc.vector.scalar_tensor_tensor(
                        out=dst[:], in0=dst[:], scalar=SLOPE, in1=dst[:],
                        op0=ALU.mult, op1=ALU.max)
                    return
                if prelu:
                    # single-op: dst = Prelu(psum + bias), alpha=0.5 via AP
                    # (bit-exact; HW ignores float-imm alpha, honors AP alpha;
                    # CoreSim lacks Prelu, hence the 2-op fallback for sim)
                    nc.scalar.activation(dst[:], psum_view[:], AF.Prelu,
                                         bias=bias_ap, alpha=acol())
                    return
                u = tmp.tile(list(psum_view.shape), F32, name=f"u{name}", tag="u")
                nc.scalar.activation(u[:], psum_view[:], AF.Identity,
                                     bias=bias_ap)
                nc.vector.scalar_tensor_tensor(
                    out=dst[:], in0=u[:], scalar=SLOPE, in1=u[:],
                    op0=ALU.mult, op1=ALU.max)

            def l1(w):
                n = 465
                x0 = xin.tile([128, 4, n], F32, name=f"x0_{w}", tag="x0")
                for a in range(4):
                    nc.sync.dma_start(out=x0[32 * a:32 * a + 32, :, :],
                                      in_=x_d[w, 32 * a:32 * a + 32, :, :])
                pt = psum4()
                for a in range(4):
                    for b in range(4):
                        mm(pt[a][32 * b:32 * b + 32, 0:n],
                           w_t[32 * a:32 * a + 32, C1IM:C1IM + 32],
                           x0[32 * a:32 * a + 32, b, :],
                           True, True, (32 * a, 32 * b))
                a1 = act.tile([128, 4, 31, 15], F32, name=f"a1_{w}", tag="a1",
                              bufs=3)
                for a in range(4):
                    evac_leaky(pt[a][:, 0:n], bcol(0), a1[:, a], f"1_{w}_{a}",
                               dve=(a % 2 == 1))
                # x+1-shifted replica on partitions 16..32 of each b-block.
                # Flat shift over the merged free dim: contiguous runs; the
                # polluted last column (x=14) is never read by L2 windows.
                a1f = a1.rearrange("p a h w -> p (a h w)")
                for b in range(4):
                    nc.sync.dma_start(
                        out=a1f[32 * b + 16:32 * b + 32, 0:1859],
                        in_=a1f[32 * b:32 * b + 16, 1:1860])
                S[w]["a1"] = a1

            def l2(w):
                _, ci, co, hi, wi, ho, wo = LAYERS[1]
                n = ho * wo
                a1 = S[w]["a1"]
                pt = psum4()
                for dy in range(2):
                    for b in range(4):
                        for a in range(4):
                            mm(pt[b][32 * a:32 * a + 32, 0:n],
                               w_t[32 * b:32 * b + 32, C2F[dy]:C2F[dy] + 32],
                               a1[32 * b:32 * b + 32, a, dy:dy + ho, 0:wo],
                               dy == 0, dy == 1, (32 * b, 32 * a))
                a2 = act.tile([128, 4, ho, wo], F32, name=f"a2_{w}", tag="a2")
                for b in range(4):
                    evac_leaky(pt[b][:, 0:n], bcol(1), a2[:, b], f"2_{w}_{b}",
                               dve=(b % 2 == 1))
                S[w]["a2"] = a2

            def l3(w, sb):
                _, ci, co, hi, wi, ho, wo = LAYERS[2]
                n = ho * wo
                a2 = S[w]["a2"]
                pt = psum4()
                for t in range(4):
                    dy, dx = t // 2, t % 2
                    for a in range(4):
                        for bb in range(2):
                            b = 2 * sb + bb
                            for h in range(2):
                                cq = 64 * bb + 32 * h
                                mm(pt[a][cq:cq + 32, 0:n],
                                   w_t[32 * a:32 * a + ci,
                                       C3T[t] + 32 * h:C3T[t] + 32 * h + 32],
                                   a2[32 * a:32 * a + ci, b,
                                      dy:dy + ho, dx:dx + wo],
                                   t == 0, t == 3, (32 * a, cq))
                a3 = pads["a3A" if sb == 0 else "a3B"][w % 3]
                for a in range(4):
                    evac_leaky(pt[a][:, 0:n], bcol(2),
                               a3[:, a, 1:1 + ho, 1:1 + wo], f"3_{w}_{sb}_{a}",
                               dve=(a % 2 == 1))
                S[w][f"a3{sb}"] = a3

            def l4(w, sb):
                _, ci, co, hi, wi, ho, wo = LAYERS[3]
                n = ho * wo
                a3 = S[w][f"a3{sb}"]
                pt = psum4()
                for t in range(4):
                    dy, dx = t // 2, t % 2
                    for a in range(4):
                        for bb in range(2):
                            for p in range(2):
                                r = 2 * bb + p
                                mm(pt[r][32 * a:32 * a + 32, 0:n],
                                   w_t[32 * r:32 * r + 32,
                                       C4T[p][t]:C4T[p][t] + 32],
                                   a3[32 * r:32 * r + 32, a,
                                      dy:dy + ho, dx:dx + wo],
                                   t == 0, t == 3, (32 * r, 32 * a))
                a4 = pads["a4"][w % 3]
                S[w]["a4"] = a4
                for bb in range(2):
                    b = 2 * sb + bb
                    t1 = tmp.tile([128, n], F32, name=f"t4_{w}_{b}", tag="u")
                    nc.scalar.activation(t1[:], pt[2 * bb + 1][:, 0:n],
                                         AF.Identity, bias=bcol(3))
                    s2 = tmp.tile([128, n], F32, name=f"s4_{w}_{b}", tag="u")
                    nc.vector.tensor_tensor(out=s2[:], in0=pt[2 * bb][:, 0:n],
                                            in1=t1[:], op=ALU.add)
                    nc.vector.scalar_tensor_tensor(
                        out=a4[:, b, 1:1 + ho, 1:1 + wo], in0=s2[:],
                        scalar=SLOPE, in1=s2[:], op0=ALU.mult, op1=ALU.max)

            def l5(w):
                _, ci, co, hi, wi, ho, wo = LAYERS[4]
                n = ho * wo
                a4 = S[w]["a4"]
                pt = psum4()
                for t in range(4):
                    dy, dx = t // 2, t % 2
                    mw = 32 if t in (0, 3) else 16
                    for a in range(4):
                        for b in range(4):
                            mm(pt[a][32 * b:32 * b + mw, 0:n],
                               w_t[32 * a:32 * a + ci, C5T[t]:C5T[t] + mw],
                               a4[32 * a:32 * a + ci, b, dy:dy + ho, dx:dx + wo],
                               t == 0, t == 3, (32 * a, 32 * b))
                a5 = pads["a5"][w % 3]
                for a in range(4):
                    evac_leaky(pt[a][:, 0:n], bcol(4),
                               a5[:, a, 1:1 + ho, 1:1 + wo], f"5_{w}_{a}",
                               dve=(a % 2 == 1))
                a5f = a5.rearrange("p a h w -> p (a h w)")
                for b in range(4):
                    nc.sync.dma_start(
                        out=a5f[32 * b + 16:32 * b + 32, 0:2243],
                        in_=a5f[32 * b:32 * b + 16, 1:2244])
                S[w]["a5"] = a5

            def l6(w):
                _, ci, co, hi, wi, ho, wo = LAYERS[5]
                n = ho * wo
                a5 = S[w]["a5"]
                pt = psum4()
                for dy in range(2):
                    for b in range(4):
                        for a in range(4):
                            mm(pt[b][32 * a:32 * a + 32, 0:n],
                               w_t[32 * b:32 * b + 32, C6F[dy]:C6F[dy] + 32],
                               a5[32 * b:32 * b + 32, a, dy:dy + ho, 0:wo],
                               dy == 0, dy == 1, (32 * b, 32 * a))
                o6 = act.tile([128, 4, ho * wo], F32, name=f"a6_{w}", tag="a6",
                              bufs=2)
                for b in range(4):
                    nc.scalar.activation(o6[:, b, :], pt[b][:, 0:n],
                                         AF.Identity, bias=bcol(5))
                    nc.sync.dma_start(out=y_d[w, b, :, :, :],
                                      in_=o6[0:128:32, b, :])

            def emit(w, st):
                if st == 0:
                    l1(w)
                elif st == 1:
                    l2(w)
                elif st in (2, 3):
                    l3(w, st - 2)
                elif st in (4, 5):
                    l4(w, st - 4)
                elif st == 6:
                    l5(w)
                else:
                    l6(w)

            NST = 8
            sched = sorted(
                ((max(w - 2, 0) if s == 0 else w + s), s, -w, w)
                for w in range(waves) for s in range(NST))
            for _, s, _, w in sched:
                emit(w, s)

    nc.compile()
    return nc


_NC_CACHE = {}


def _get_nc():
    if "nc" not in _NC_CACHE:
        _NC_CACHE["nc"] = build()
    return _NC_CACHE["nc"]


def _post_process(Rx, M):
    import jax
    import jax.numpy as jnp

    cpu = jax.devices("cpu")[0]
    with jax.default_device(cpu):
        n = NSEN
        Rx = jnp.asarray(Rx)
        K = Rx[:, :n, :] + 1j * Rx[:, n:, :]
        g = jnp.einsum("bij,bik->bjk", jnp.conj(K), K)
        Rz = g + EPS * jnp.eye(n, dtype=g.dtype)

        w, v = jnp.linalg.eig(Rz)
        order = jnp.argsort(jnp.abs(w), axis=-1)[:, ::-1]
        Un = jnp.take_along_axis(v, order[:, None, :], axis=2)[:, :, M:]
        F = jnp.einsum("bik,bjk->bij", Un, jnp.conj(Un))
        ids = (jnp.arange(n)[None, :] - jnp.arange(n)[:, None] + n - 1).ravel()
        seg = lambda f: jax.ops.segment_sum(f.ravel(), ids, num_segments=2 * n - 1)
        coeff = jax.vmap(seg)(F)
        L = 2 * n - 2
        A = jnp.broadcast_to(jnp.eye(L, k=-1, dtype=coeff.dtype),
                             (coeff.shape[0], L, L))
        A = A.at[:, 0, :].set(-coeff[:, 1:] / coeff[:, :1])
        roots = jnp.linalg.eig(A)[0]
        inv_pi = 1.0 / np.pi
        DOA_all = jnp.arcsin(inv_pi * jnp.angle(roots))
        o2 = jnp.argsort(jnp.abs(jnp.abs(roots) - 1.0), axis=-1)
        roots_s = jnp.take_along_axis(roots, o2, axis=-1)
        sorted_angels = jnp.arcsin(inv_pi * jnp.angle(roots_s))
        inside = (jnp.abs(roots_s) - 1.0) < 0
        o3 = jnp.argsort((~inside).astype(jnp.int32), axis=-1, stable=True)[:, :M]
        roots_in = jnp.take_along_axis(roots_s, o3, axis=-1)
        DOA = jnp.arcsin(inv_pi * jnp.angle(roots_in))
        return (np.asarray(DOA), np.asarray(DOA_all), np.asarray(roots_s[-1]),
                np.asarray(Rz), np.asarray(sorted_angels))


def run_cnn_device(New_Rx_tau, params, trace=False):
    """Run the CNN on the 8 NeuronCores; returns Rx [B, 32, 16] float32."""
    nc = _get_nc()
    wblob = _weights_blob(params)
    x = np.asarray(New_Rx_tau, np.float32)
    in_maps = []
    for c in range(N_CORES):
        in_maps.append({"x": _pack_x(x[c * BPC:(c + 1) * BPC]), "w": wblob})
    res = run_bass_kernel_spmd(nc, in_maps, list(range(N_CORES)), trace=trace)
    _NC_CACHE["last_res"] = res
    return np.concatenate(
        [_unpack_y(res.results[c]["y"]) for c in range(N_CORES)], axis=0)


def kernel(New_Rx_tau, M, conv1_w, conv1_b, conv2_w, conv2_b, conv3_w, conv3_b,
           deconv2_w, deconv2_b, deconv3_w, deconv3_b, deconv4_w, deconv4_b):
    params = dict(conv1_w=np.asarray(conv1_w), conv1_b=np.asarray(conv1_b),
                  conv2_w=np.asarray(conv2_w), conv2_b=np.asarray(conv2_b),
                  conv3_w=np.asarray(conv3_w), conv3_b=np.asarray(conv3_b),
                  deconv2_w=np.asarray(deconv2_w), deconv2_b=np.asarray(deconv2_b),
                  deconv3_w=np.asarray(deconv3_w), deconv3_b=np.asarray(deconv3_b),
                  deconv4_w=np.asarray(deconv4_w), deconv4_b=np.asarray(deconv4_b))
    Rx = run_cnn_device(np.asarray(New_Rx_tau), params)
    return _post_process(Rx, int(M))


def golden_cnn(x, p):
    """Numpy float64 reference of the CNN (debugging aid)."""
    def lrelu(v):
        return np.where(v >= 0, v, SLOPE * v)

    def conv(v, w, b):
        co, ci, _, _ = w.shape
        B, _, h, ww = v.shape
        out = np.zeros((B, co, h - 1, ww - 1), v.dtype)
        for dy in range(2):
            for dx in range(2):
                out += np.einsum("bihw,oi->bohw",
                                 v[:, :, dy:dy + h - 1, dx:dx + ww - 1],
                                 w[:, :, dy, dx])
        return out + b[None, :, None, None]

    def deconv(v, w, b):
        ci, co, _, _ = w.shape
        B, _, h, ww = v.shape
        out = np.zeros((B, co, h + 1, ww + 1), v.dtype)
        for dy in range(2):
            for dx in range(2):
                out[:, :, 1 - dy:1 - dy + h, 1 - dx:1 - dx + ww] += np.einsum(
                    "bihw,io->bohw", v, w[:, :, 1 - dy, 1 - dx])
        return out + b[None, :, None, None]

    x = x.astype(np.float64)
    p64 = {k: v.astype(np.float64) for k, v in p.items()}
    v = lrelu(conv(x, p64["conv1_w"], p64["conv1_b"]))
    v = lrelu(conv(v, p64["conv2_w"], p64["conv2_b"]))
    v = lrelu(conv(v, p64["conv3_w"], p64["conv3_b"]))
    v = lrelu(deconv(v, p64["deconv2_w"], p64["deconv2_b"]))
    v = lrelu(deconv(v, p64["deconv3_w"], p64["deconv3_b"]))
    v = deconv(v, p64["deconv4_w"], p64["deconv4_b"])
    return v[:, 0]


# revision 16
# speedup vs baseline: 1.0607x; 1.0607x over previous
"""Trainium2 kernel for nn_Deep_Root_Net.

Device (8 NeuronCores, pure data parallel, 128 samples/core):
  The 6-layer CNN (3x conv k2 VALID + 3x convT k2 stride1) runs on the PE
  array in 32x32 tiling mode: each sample-layer is a chain of PSUM-
  accumulated shifted-tap matmuls; 16 chains run concurrently across the
  16 PE sub-tiles. L1 is a single matmul per sample (taps folded into
  K=32 via host-side im2col). Biases are folded into the PSUM evacuation
  (ACT Identity with a per-partition bias column). LeakyReLU(0.5) =
  ACT copy, then DVE max(0.5*u, u) (bit-exact vs jnp.where).
  ConvT layers read zero-ring-padded SBUF activations so every tap
  covers the full contiguous output region.

Host: gramian + Root-MUSIC (two LAPACK eigs per sample) mirror the
reference implementation verbatim on CPU jax - the eig deflation order and
stable argsort semantics are LAPACK/jax-specific and chaotically sensitive
(a 1e-7 perturbation of Rx flips root orderings O(1)), so the only sound
implementation runs the identical code path on the numerically stable
part of the pipeline (Rx -> Rz is stable; the CNN on device matches a
float64 reference to ~3e-8).
"""
import os
import sys

import numpy as np

for _p in ("/opt/trn_rl_repo", "/root/.axon_site/_ro/trn_rl_repo"):
    if os.path.isdir(_p) and _p not in sys.path:
        sys.path.append(_p)

import concourse.bacc as bacc
import concourse.mybir as mybir
import concourse.tile as tile
from concourse.bass_utils import run_bass_kernel_spmd

F32 = mybir.dt.float32
AF = mybir.ActivationFunctionType
ALU = mybir.AluOpType

N_CORES = 8
BPC = 128      # samples per core
WAVES = 8      # 16 samples per wave
SLOPE = 0.5
EPS = 1.0
NSEN = 16

# kind, Cin, Cout, Hin, Win, Hout, Wout
LAYERS = [
    ("conv",    8, 16, 32, 16, 31, 15),
    ("conv",   16, 32, 31, 15, 30, 14),
    ("conv",   32, 64, 30, 14, 29, 13),
    ("deconv", 64, 32, 29, 13, 30, 14),
    ("deconv", 32, 16, 30, 14, 31, 15),
    ("deconv", 16,  1, 31, 15, 32, 16),
]

# weights blob column map. Tap blocks are per-32-row-quadrant (replicated
# x4); bias columns are full-height [128, 1]. L2/L6 fold the dx taps into
# K=32 (activations carry a x+1-shifted replica on partitions 16..32).
C1IM = 0                                     # L1 im2col lhsT [32, 32] (M pad)
C2F = [32, 64]                               # L2 dy-folded lhsT [32, 32]
C3T = [96, 160, 224, 288]                    # taps [32,64]
C4T = [[352 + 128 * h + 32 * t for t in range(4)] for h in range(2)]
C5T = [608, 640, 656, 672]                   # taps [32,16] (t0,t3 pad to 32)
C6F = [704, 736]                             # L6 dy-folded lhsT [32, 32] (M pad)
BCOL = 768                                   # 6 bias columns [128, 1]
ACOL = 774                                   # alpha column (0.5) [128, 1]
WCOLS = 800


def _weights_blob(p):
    q = np.zeros((32, WCOLS), np.float32)

    def conv_tap(w, dy, dx):       # torch conv w [Cout, Cin, 2, 2] -> [Cin, Cout]
        return np.ascontiguousarray(w[:, :, dy, dx].T)

    def deconv_tap(w, dy, dx):     # torch convT w [Cin, Cout, 2, 2] -> [Cin, Cout]
        return np.ascontiguousarray(w[:, :, 1 - dy, 1 - dx])

    for t in range(4):
        dy, dx = t // 2, t % 2
        q[8 * t:8 * t + 8, C1IM:C1IM + 16] = conv_tap(p["conv1_w"], dy, dx)
        q[16 * dx:16 * dx + 16, C2F[dy]:C2F[dy] + 32] = \
            conv_tap(p["conv2_w"], dy, dx)
        q[0:32, C3T[t]:C3T[t] + 64] = conv_tap(p["conv3_w"], dy, dx)
        for h in range(2):
            q[0:32, C4T[h][t]:C4T[h][t] + 32] = \
                deconv_tap(p["deconv2_w"], dy, dx)[32 * h:32 * h + 32]
        q[0:32, C5T[t]:C5T[t] + 16] = deconv_tap(p["deconv3_w"], dy, dx)
        q[16 * dx:16 * dx + 16, C6F[dy]:C6F[dy] + 1] = \
            deconv_tap(p["deconv4_w"], dy, dx)
    blob = np.tile(q, (4, 1))

    r = np.arange(128)
    bias = np.zeros((128, 6), np.float32)
    c32, c64 = r % 32, r % 64
    bias[:, 0] = np.where(c32 < 16, p["conv1_b"][np.minimum(c32, 15)], 0)
    bias[:, 1] = p["conv2_b"][c32]
    bias[:, 2] = p["conv3_b"][c64]
    bias[:, 3] = p["deconv2_b"][c32]
    bias[:, 4] = np.where(c32 < 16, p["deconv3_b"][np.minimum(c32, 15)], 0)
    bias[:, 5] = np.where(c32 == 0, p["deconv4_b"][0], 0)
    blob[:, BCOL:BCOL + 6] = bias
    blob[:, ACOL] = SLOPE
    return blob


def _pack_x(x_core):
    # im2col: [128, 8, 32, 16] -> [WAVES, 4(a), 32(t*8+c), 4(b), 465]
    # sample = w*16 + 4a + b; row t*8+c = x[c, dy:dy+31, dx:dx+15].ravel()
    xim = np.empty((BPC, 4, 8, 465), np.float32)
    for t in range(4):
        dy, dx = t // 2, t % 2
        xim[:, t] = x_core[:, :, dy:dy + 31, dx:dx + 15].reshape(BPC, 8, 465)
    xim = xim.reshape(BPC, 32, 465)
    xp = np.empty((WAVES, 4, 32, 4, 465), np.float32)
    for w in range(WAVES):
        for a in range(4):
            for b in range(4):
                xp[w, a, :, b] = xim[w * 16 + 4 * a + b]
    return xp.reshape(WAVES, 128, 4, 465)


def _unpack_y(y):
    # y [WAVES, 4(b), 4(a), 32, 16] -> [128, 32, 16]
    rx = np.empty((BPC, 32, 16), np.float32)
    for w in range(WAVES):
        for b in range(4):
            for a in range(4):
                rx[w * 16 + 4 * a + b] = y[w, b, a]
    return rx


def build(waves=WAVES, prelu=True):
    nc = bacc.Bacc("TRN2", target_bir_lowering=False, debug=False)
    x_d = nc.dram_tensor("x", [waves, 128, 4, 465], F32, kind="ExternalInput")
    w_d = nc.dram_tensor("w", [128, WCOLS], F32, kind="ExternalInput")
    y_d = nc.dram_tensor("y", [waves, 4, 4, 32, 16], F32, kind="ExternalOutput")

    with tile.TileContext(nc) as tc:
        with tc.tile_pool(name="wt", bufs=1) as wt, \
             tc.tile_pool(name="xin", bufs=3) as xin, \
             tc.tile_pool(name="act", bufs=3) as act, \
             tc.tile_pool(name="tmp", bufs=6) as tmp, \
             tc.tile_pool(name="ps", bufs=8, space="PSUM") as ps:

            w_t = wt.tile([128, WCOLS], F32, name="wblob")
            nc.sync.dma_start(out=w_t[:, :], in_=w_d[:, :])

            def bcol(i):
                return w_t[:, BCOL + i:BCOL + i + 1]

            def acol():
                return w_t[:, ACOL:ACOL + 1]

            S = [dict() for _ in range(waves)]

            # persistent zero-ring-padded activation buffers (3-deep ring
            # per layer, indexed w%3): rings are zeroed once here; waves
            # overwrite interiors only, so no per-wave memsets.
            pads = {}
            for key, shape in (("a3A", [128, 4, 31, 15]),
                               ("a3B", [128, 4, 31, 15]),
                               ("a4", [128, 4, 32, 16]),
                               ("a5", [128, 4, 33, 17])):
                tiles = []
                for i in range(3):
                    tl = act.tile(shape, F32, name=f"pad_{key}_{i}",
                                  tag=f"pad_{key}_{i}", bufs=1)
                    nc.gpsimd.memset(tl[:], 0.0)
                    tiles.append(tl)
                pads[key] = tiles

            def mm(out, lhsT, rhs, start, stop, pos):
                # skip_group_check: CoreSim's global zero-region bookkeeping
                # mis-addresses partition offsets encoded in psum AP offsets;
                # the per-tile pending-zero semantics (and HW) handle
                # per-partition accumulation chains correctly.
                nc.tensor.matmul(out=out, lhsT=lhsT, rhs=rhs, start=start,
                                 stop=stop, tile_position=pos,
                                 skip_group_check=True)

            def psum4():
                return [ps.tile([128, 512], F32, name=f"ps{nc.next_id()}",
                                tag="ps") for _ in range(4)]

            def evac_leaky(psum_view, bias_ap, dst, name, dve=False):
                if dve:
                    # DVE path: u = psum + bias, then in-place max(0.5u, u).
                    # Splitting evacs across ACT/DVE halves the per-stage
                    # PSUM-release latency (banks evacuate in parallel).
                    nc.vector.tensor_scalar_add(out=dst[:], in0=psum_view[:],
                                                scalar1=bias_ap)
                    nc.vector.scalar_tensor_tensor(
                        out=dst[:], in0=dst[:], scalar=SLOPE, in1=dst[:],
                        op0=ALU.mult, op1=ALU.max)
                    return
                if prelu:
                    # single-op: dst = Prelu(psum + bias), alpha=0.5 via AP
                    # (bit-exact; HW ignores float-imm alpha, honors AP alpha;
                    # CoreSim lacks Prelu, hence the 2-op fallback for sim)
                    nc.scalar.activation(dst[:], psum_view[:], AF.Prelu,
                                         bias=bias_ap, alpha=acol())
                    return
                u = tmp.tile(list(psum_view.shape), F32, name=f"u{name}", tag="u")
                nc.scalar.activation(u[:], psum_view[:], AF.Identity,
                                     bias=bias_ap)
                nc.vector.scalar_tensor_tensor(
                    out=dst[:], in0=u[:], scalar=SLOPE, in1=u[:],
                    op0=ALU.mult, op1=ALU.max)

            def l1(w):
                n = 465
                x0 = xin.tile([128, 4, n], F32, name=f"x0_{w}", tag="x0")
                nc.sync.dma_start(out=x0[:, :, :], in_=x_d[w, :, :, :])
                pt = psum4()
                for a in range(4):
                    for b in range(4):
                        mm(pt[a][32 * b:32 * b + 32, 0:n],
                           w_t[32 * a:32 * a + 32, C1IM:C1IM + 32],
                           x0[32 * a:32 * a + 32, b, :],
                           True, True, (32 * a, 32 * b))
                a1 = act.tile([128, 4, 31, 15], F32, name=f"a1_{w}", tag="a1",
                              bufs=3)
                for a in range(4):
                    evac_leaky(pt[a][:, 0:n], bcol(0), a1[:, a], f"1_{w}_{a}",
                               dve=(a % 2 == 1))
                # x+1-shifted replica on partitions 16..32 of each b-block.
                # Flat shift over the merged free dim: contiguous runs; the
                # polluted last column (x=14) is never read by L2 windows.
                a1f = a1.rearrange("p a h w -> p (a h w)")
                for b in range(4):
                    nc.sync.dma_start(
                        out=a1f[32 * b + 16:32 * b + 32, 0:1859],
                        in_=a1f[32 * b:32 * b + 16, 1:1860])
                S[w]["a1"] = a1

            def l2(w):
                _, ci, co, hi, wi, ho, wo = LAYERS[1]
                n = ho * wo
                a1 = S[w]["a1"]
                pt = psum4()
                for dy in range(2):
                    for b in range(4):
                        for a in range(4):
                            mm(pt[b][32 * a:32 * a + 32, 0:n],
                               w_t[32 * b:32 * b + 32, C2F[dy]:C2F[dy] + 32],
                               a1[32 * b:32 * b + 32, a, dy:dy + ho, 0:wo],
                               dy == 0, dy == 1, (32 * b, 32 * a))
                a2 = act.tile([128, 4, ho, wo], F32, name=f"a2_{w}", tag="a2")
                for b in range(4):
                    evac_leaky(pt[b][:, 0:n], bcol(1), a2[:, b], f"2_{w}_{b}",
                               dve=(b % 2 == 1))
                S[w]["a2"] = a2

            def l3(w, sb):
                _, ci, co, hi, wi, ho, wo = LAYERS[2]
                n = ho * wo
                a2 = S[w]["a2"]
                pt = psum4()
                for t in range(4):
                    dy, dx = t // 2, t % 2
                    for a in range(4):
                        for bb in range(2):
                            b = 2 * sb + bb
                            for h in range(2):
                                cq = 64 * bb + 32 * h
                                mm(pt[a][cq:cq + 32, 0:n],
                                   w_t[32 * a:32 * a + ci,
                                       C3T[t] + 32 * h:C3T[t] + 32 * h + 32],
                                   a2[32 * a:32 * a + ci, b,
                                      dy:dy + ho, dx:dx + wo],
                                   t == 0, t == 3, (32 * a, cq))
                a3 = pads["a3A" if sb == 0 else "a3B"][w % 3]
                for a in range(4):
                    evac_leaky(pt[a][:, 0:n], bcol(2),
                               a3[:, a, 1:1 + ho, 1:1 + wo], f"3_{w}_{sb}_{a}",
                               dve=(a % 2 == 1))
                S[w][f"a3{sb}"] = a3

            def l4(w, sb):
                _, ci, co, hi, wi, ho, wo = LAYERS[3]
                n = ho * wo
                a3 = S[w][f"a3{sb}"]
                pt = psum4()
                for t in range(4):
                    dy, dx = t // 2, t % 2
                    for a in range(4):
                        for bb in range(2):
                            for p in range(2):
                                r = 2 * bb + p
                                mm(pt[r][32 * a:32 * a + 32, 0:n],
                                   w_t[32 * r:32 * r + 32,
                                       C4T[p][t]:C4T[p][t] + 32],
                                   a3[32 * r:32 * r + 32, a,
                                      dy:dy + ho, dx:dx + wo],
                                   t == 0, t == 3, (32 * r, 32 * a))
                a4 = pads["a4"][w % 3]
                S[w]["a4"] = a4
                for bb in range(2):
                    b = 2 * sb + bb
                    t1 = tmp.tile([128, n], F32, name=f"t4_{w}_{b}", tag="u")
                    nc.scalar.activation(t1[:], pt[2 * bb + 1][:, 0:n],
                                         AF.Identity, bias=bcol(3))
                    s2 = tmp.tile([128, n], F32, name=f"s4_{w}_{b}", tag="u")
                    nc.vector.tensor_tensor(out=s2[:], in0=pt[2 * bb][:, 0:n],
                                            in1=t1[:], op=ALU.add)
                    nc.vector.scalar_tensor_tensor(
                        out=a4[:, b, 1:1 + ho, 1:1 + wo], in0=s2[:],
                        scalar=SLOPE, in1=s2[:], op0=ALU.mult, op1=ALU.max)

            def l5(w):
                _, ci, co, hi, wi, ho, wo = LAYERS[4]
                n = ho * wo
                a4 = S[w]["a4"]
                pt = psum4()
                for t in range(4):
                    dy, dx = t // 2, t % 2
                    mw = 32 if t in (0, 3) else 16
                    for a in range(4):
                        for b in range(4):
                            mm(pt[a][32 * b:32 * b + mw, 0:n],
                               w_t[32 * a:32 * a + ci, C5T[t]:C5T[t] + mw],
                               a4[32 * a:32 * a + ci, b, dy:dy + ho, dx:dx + wo],
                               t == 0, t == 3, (32 * a, 32 * b))
                a5 = pads["a5"][w % 3]
                for a in range(4):
                    evac_leaky(pt[a][:, 0:n], bcol(4),
                               a5[:, a, 1:1 + ho, 1:1 + wo], f"5_{w}_{a}",
                               dve=(a % 2 == 1))
                a5f = a5.rearrange("p a h w -> p (a h w)")
                for b in range(4):
                    nc.sync.dma_start(
                        out=a5f[32 * b + 16:32 * b + 32, 0:2243],
                        in_=a5f[32 * b:32 * b + 16, 1:2244])
                S[w]["a5"] = a5

            def l6(w):
                _, ci, co, hi, wi, ho, wo = LAYERS[5]
                n = ho * wo
                a5 = S[w]["a5"]
                pt = psum4()
                for dy in range(2):
                    for b in range(4):
                        for a in range(4):
                            mm(pt[b][32 * a:32 * a + 32, 0:n],
                               w_t[32 * b:32 * b + 32, C6F[dy]:C6F[dy] + 32],
                               a5[32 * b:32 * b + 32, a, dy:dy + ho, 0:wo],
                               dy == 0, dy == 1, (32 * b, 32 * a))
                o6 = act.tile([128, 4, ho * wo], F32, name=f"a6_{w}", tag="a6",
                              bufs=2)
                for b in range(4):
                    nc.scalar.activation(o6[:, b, :], pt[b][:, 0:n],
                                         AF.Identity, bias=bcol(5))
                    nc.sync.dma_start(out=y_d[w, b, :, :, :],
                                      in_=o6[0:128:32, b, :])

            def emit(w, st):
                if st == 0:
                    l1(w)
                elif st == 1:
                    l2(w)
                elif st in (2, 3):
                    l3(w, st - 2)
                elif st in (4, 5):
                    l4(w, st - 4)
                elif st == 6:
                    l5(w)
                else:
                    l6(w)

            NST = 8
            sched = sorted(
                ((max(w - 2, 0) if s == 0 else w + s), s, -w, w)
                for w in range(waves) for s in range(NST))
            for _, s, _, w in sched:
                emit(w, s)

    nc.compile()
    return nc


_NC_CACHE = {}


def _get_nc():
    if "nc" not in _NC_CACHE:
        _NC_CACHE["nc"] = build()
    return _NC_CACHE["nc"]


def _post_process(Rx, M):
    import jax
    import jax.numpy as jnp

    cpu = jax.devices("cpu")[0]
    with jax.default_device(cpu):
        n = NSEN
        Rx = jnp.asarray(Rx)
        K = Rx[:, :n, :] + 1j * Rx[:, n:, :]
        g = jnp.einsum("bij,bik->bjk", jnp.conj(K), K)
        Rz = g + EPS * jnp.eye(n, dtype=g.dtype)

        w, v = jnp.linalg.eig(Rz)
        order = jnp.argsort(jnp.abs(w), axis=-1)[:, ::-1]
        Un = jnp.take_along_axis(v, order[:, None, :], axis=2)[:, :, M:]
        F = jnp.einsum("bik,bjk->bij", Un, jnp.conj(Un))
        ids = (jnp.arange(n)[None, :] - jnp.arange(n)[:, None] + n - 1).ravel()
        seg = lambda f: jax.ops.segment_sum(f.ravel(), ids, num_segments=2 * n - 1)
        coeff = jax.vmap(seg)(F)
        L = 2 * n - 2
        A = jnp.broadcast_to(jnp.eye(L, k=-1, dtype=coeff.dtype),
                             (coeff.shape[0], L, L))
        A = A.at[:, 0, :].set(-coeff[:, 1:] / coeff[:, :1])
        roots = jnp.linalg.eig(A)[0]
        inv_pi = 1.0 / np.pi
        DOA_all = jnp.arcsin(inv_pi * jnp.angle(roots))
        o2 = jnp.argsort(jnp.abs(jnp.abs(roots) - 1.0), axis=-1)
        roots_s = jnp.take_along_axis(roots, o2, axis=-1)
        sorted_angels = jnp.arcsin(inv_pi * jnp.angle(roots_s))
        inside = (jnp.abs(roots_s) - 1.0) < 0
        o3 = jnp.argsort((~inside).astype(jnp.int32), axis=-1, stable=True)[:, :M]
        roots_in = jnp.take_along_axis(roots_s, o3, axis=-1)
        DOA = jnp.arcsin(inv_pi * jnp.angle(roots_in))
        return (np.asarray(DOA), np.asarray(DOA_all), np.asarray(roots_s[-1]),
                np.asarray(Rz), np.asarray(sorted_angels))


def run_cnn_device(New_Rx_tau, params, trace=False):
    """Run the CNN on the 8 NeuronCores; returns Rx [B, 32, 16] float32."""
    nc = _get_nc()
    wblob = _weights_blob(params)
    x = np.asarray(New_Rx_tau, np.float32)
    in_maps = []
    for c in range(N_CORES):
        in_maps.append({"x": _pack_x(x[c * BPC:(c + 1) * BPC]), "w": wblob})
    res = run_bass_kernel_spmd(nc, in_maps, list(range(N_CORES)), trace=trace)
    _NC_CACHE["last_res"] = res
    return np.concatenate(
        [_unpack_y(res.results[c]["y"]) for c in range(N_CORES)], axis=0)


def kernel(New_Rx_tau, M, conv1_w, conv1_b, conv2_w, conv2_b, conv3_w, conv3_b,
           deconv2_w, deconv2_b, deconv3_w, deconv3_b, deconv4_w, deconv4_b):
    params = dict(conv1_w=np.asarray(conv1_w), conv1_b=np.asarray(conv1_b),
                  conv2_w=np.asarray(conv2_w), conv2_b=np.asarray(conv2_b),
                  conv3_w=np.asarray(conv3_w), conv3_b=np.asarray(conv3_b),
                  deconv2_w=np.asarray(deconv2_w), deconv2_b=np.asarray(deconv2_b),
                  deconv3_w=np.asarray(deconv3_w), deconv3_b=np.asarray(deconv3_b),
                  deconv4_w=np.asarray(deconv4_w), deconv4_b=np.asarray(deconv4_b))
    Rx = run_cnn_device(np.asarray(New_Rx_tau), params)
    return _post_process(Rx, int(M))


def golden_cnn(x, p):
    """Numpy float64 reference of the CNN (debugging aid)."""
    def lrelu(v):
        return np.where(v >= 0, v, SLOPE * v)

    def conv(v, w, b):
        co, ci, _, _ = w.shape
        B, _, h, ww = v.shape
        out = np.zeros((B, co, h - 1, ww - 1), v.dtype)
        for dy in range(2):
            for dx in range(2):
                out += np.einsum("bihw,oi->bohw",
                                 v[:, :, dy:dy + h - 1, dx:dx + ww - 1],
                                 w[:, :, dy, dx])
        return out + b[None, :, None, None]

    def deconv(v, w, b):
        ci, co, _, _ = w.shape
        B, _, h, ww = v.shape
        out = np.zeros((B, co, h + 1, ww + 1), v.dtype)
        for dy in range(2):
            for dx in range(2):
                out[:, :, 1 - dy:1 - dy + h, 1 - dx:1 - dx + ww] += np.einsum(
                    "bihw,io->bohw", v, w[:, :, 1 - dy, 1 - dx])
        return out + b[None, :, None, None]

    x = x.astype(np.float64)
    p64 = {k: v.astype(np.float64) for k, v in p.items()}
    v = lrelu(conv(x, p64["conv1_w"], p64["conv1_b"]))
    v = lrelu(conv(v, p64["conv2_w"], p64["conv2_b"]))
    v = lrelu(conv(v, p64["conv3_w"], p64["conv3_b"]))
    v = lrelu(deconv(v, p64["deconv2_w"], p64["deconv2_b"]))
    v = lrelu(deconv(v, p64["deconv3_w"], p64["deconv3_b"]))
    v = deconv(v, p64["deconv4_w"], p64["deconv4_b"])
    return v[:, 0]


# revision 18
# speedup vs baseline: 1.3128x; 1.2376x over previous
"""Trainium2 kernel for nn_Deep_Root_Net.

Device (8 NeuronCores, pure data parallel, 128 samples/core):
  The 6-layer CNN (3x conv k2 VALID + 3x convT k2 stride1) runs on the PE
  array in 32x32 tiling mode: each sample-layer is a chain of PSUM-
  accumulated shifted-tap matmuls; 16 chains run concurrently across the
  16 PE sub-tiles. L1 is a single matmul per sample (taps folded into
  K=32 via host-side im2col). Biases are folded into the PSUM evacuation
  (ACT Identity with a per-partition bias column). LeakyReLU(0.5) =
  ACT copy, then DVE max(0.5*u, u) (bit-exact vs jnp.where).
  ConvT layers read zero-ring-padded SBUF activations so every tap
  covers the full contiguous output region.

Host: gramian + Root-MUSIC (two LAPACK eigs per sample) mirror the
reference implementation verbatim on CPU jax - the eig deflation order and
stable argsort semantics are LAPACK/jax-specific and chaotically sensitive
(a 1e-7 perturbation of Rx flips root orderings O(1)), so the only sound
implementation runs the identical code path on the numerically stable
part of the pipeline (Rx -> Rz is stable; the CNN on device matches a
float64 reference to ~3e-8).
"""
import os
import sys

import numpy as np

for _p in ("/opt/trn_rl_repo", "/root/.axon_site/_ro/trn_rl_repo"):
    if os.path.isdir(_p) and _p not in sys.path:
        sys.path.append(_p)

import concourse.bacc as bacc
import concourse.mybir as mybir
import concourse.tile as tile
from concourse.bass_utils import run_bass_kernel_spmd

F32 = mybir.dt.float32
AF = mybir.ActivationFunctionType
ALU = mybir.AluOpType

N_CORES = 8
BPC = 128      # samples per core
WAVES = 8      # 16 samples per wave
SLOPE = 0.5
EPS = 1.0
NSEN = 16

# kind, Cin, Cout, Hin, Win, Hout, Wout
LAYERS = [
    ("conv",    8, 16, 32, 16, 31, 15),
    ("conv",   16, 32, 31, 15, 30, 14),
    ("conv",   32, 64, 30, 14, 29, 13),
    ("deconv", 64, 32, 29, 13, 30, 14),
    ("deconv", 32, 16, 30, 14, 31, 15),
    ("deconv", 16,  1, 31, 15, 32, 16),
]

# weights blob column map. Tap blocks are per-32-row-quadrant (replicated
# x4); bias columns are full-height [128, 1]. L2/L6 fold the dx taps into
# K=32 (activations carry a x+1-shifted replica on partitions 16..32).
C1IM = 0                                     # L1 im2col lhsT [32, 32] (M pad)
C2F = [32, 64]                               # L2 dy-folded lhsT [32, 32]
C3T = [96, 160, 224, 288]                    # taps [32,64]
C4F = [352 + 32 * t for t in range(4)]        # L4 stacked lhsT [64, 32]
C5T = [608, 640, 656, 672]                   # taps [32,16] (t0,t3 pad to 32)
C6F = [704, 736]                             # L6 dy-folded lhsT [32, 32] (M pad)
BCOL = 768                                   # 6 bias columns [128, 1]
ACOL = 774                                   # alpha column (0.5) [128, 1]
WCOLS = 800


def _weights_blob(p):
    q = np.zeros((32, WCOLS), np.float32)

    def conv_tap(w, dy, dx):       # torch conv w [Cout, Cin, 2, 2] -> [Cin, Cout]
        return np.ascontiguousarray(w[:, :, dy, dx].T)

    def deconv_tap(w, dy, dx):     # torch convT w [Cin, Cout, 2, 2] -> [Cin, Cout]
        return np.ascontiguousarray(w[:, :, 1 - dy, 1 - dx])

    for t in range(4):
        dy, dx = t // 2, t % 2
        q[8 * t:8 * t + 8, C1IM:C1IM + 16] = conv_tap(p["conv1_w"], dy, dx)
        q[16 * dx:16 * dx + 16, C2F[dy]:C2F[dy] + 32] = \
            conv_tap(p["conv2_w"], dy, dx)
        q[0:32, C3T[t]:C3T[t] + 64] = conv_tap(p["conv3_w"], dy, dx)
        q[0:32, C5T[t]:C5T[t] + 16] = deconv_tap(p["deconv3_w"], dy, dx)
        q[16 * dx:16 * dx + 16, C6F[dy]:C6F[dy] + 1] = \
            deconv_tap(p["deconv4_w"], dy, dx)
    blob = np.tile(q, (4, 1))
    for t in range(4):
        dy, dx = t // 2, t % 2
        w4 = deconv_tap(p["deconv2_w"], dy, dx)          # [64, 32]
        blob[0:64, C4F[t]:C4F[t] + 32] = w4
        blob[64:128, C4F[t]:C4F[t] + 32] = w4

    r = np.arange(128)
    bias = np.zeros((128, 6), np.float32)
    c32, c64 = r % 32, r % 64
    bias[:, 0] = np.where(c32 < 16, p["conv1_b"][np.minimum(c32, 15)], 0)
    bias[:, 1] = p["conv2_b"][c32]
    bias[:, 2] = p["conv3_b"][c64]
    bias[:, 3] = p["deconv2_b"][c32]
    bias[:, 4] = np.where(c32 < 16, p["deconv3_b"][np.minimum(c32, 15)], 0)
    bias[:, 5] = np.where(c32 == 0, p["deconv4_b"][0], 0)
    blob[:, BCOL:BCOL + 6] = bias
    blob[:, ACOL] = SLOPE
    return blob


def _pack_x(x_core):
    # im2col: [128, 8, 32, 16] -> [WAVES, 4(a), 32(t*8+c), 4(b), 465]
    # sample = w*16 + 4a + b; row t*8+c = x[c, dy:dy+31, dx:dx+15].ravel()
    xim = np.empty((BPC, 4, 8, 465), np.float32)
    for t in range(4):
        dy, dx = t // 2, t % 2
        xim[:, t] = x_core[:, :, dy:dy + 31, dx:dx + 15].reshape(BPC, 8, 465)
    xim = xim.reshape(BPC, 32, 465)
    xp = np.empty((WAVES, 4, 32, 4, 465), np.float32)
    for w in range(WAVES):
        for a in range(4):
            for b in range(4):
                xp[w, a, :, b] = xim[w * 16 + 4 * a + b]
    return xp.reshape(WAVES, 128, 4, 465)


def _unpack_y(y):
    # y [WAVES, 4(b), 4(a), 32, 16] -> [128, 32, 16]
    rx = np.empty((BPC, 32, 16), np.float32)
    for w in range(WAVES):
        for b in range(4):
            for a in range(4):
                rx[w * 16 + 4 * a + b] = y[w, b, a]
    return rx


def build(waves=WAVES, prelu=True):
    nc = bacc.Bacc("TRN2", target_bir_lowering=False, debug=False)
    x_d = nc.dram_tensor("x", [waves, 128, 4, 465], F32, kind="ExternalInput")
    w_d = nc.dram_tensor("w", [128, WCOLS], F32, kind="ExternalInput")
    y_d = nc.dram_tensor("y", [waves, 4, 4, 32, 16], F32, kind="ExternalOutput")

    with tile.TileContext(nc) as tc:
        with tc.tile_pool(name="wt", bufs=1) as wt, \
             tc.tile_pool(name="xin", bufs=3) as xin, \
             tc.tile_pool(name="act", bufs=3) as act, \
             tc.tile_pool(name="tmp", bufs=6) as tmp, \
             tc.tile_pool(name="ps", bufs=8, space="PSUM") as ps:

            w_t = wt.tile([128, WCOLS], F32, name="wblob")
            nc.sync.dma_start(out=w_t[:, :], in_=w_d[:, :])

            def bcol(i):
                return w_t[:, BCOL + i:BCOL + i + 1]

            def acol():
                return w_t[:, ACOL:ACOL + 1]

            S = [dict() for _ in range(waves)]

            # persistent zero-ring-padded activation buffers (3-deep ring
            # per layer, indexed w%3): rings are zeroed once here; waves
            # overwrite interiors only, so no per-wave memsets.
            pads = {}
            for key, shape in (("a3A", [128, 4, 31, 15]),
                               ("a3B", [128, 4, 31, 15]),
                               ("a4", [128, 4, 32, 16]),
                               ("a5", [128, 4, 33, 17])):
                tiles = []
                for i in range(3):
                    tl = act.tile(shape, F32, name=f"pad_{key}_{i}",
                                  tag=f"pad_{key}_{i}", bufs=1)
                    nc.gpsimd.memset(tl[:], 0.0)
                    tiles.append(tl)
                pads[key] = tiles

            def mm(out, lhsT, rhs, start, stop, pos):
                # skip_group_check: CoreSim's global zero-region bookkeeping
                # mis-addresses partition offsets encoded in psum AP offsets;
                # the per-tile pending-zero semantics (and HW) handle
                # per-partition accumulation chains correctly.
                nc.tensor.matmul(out=out, lhsT=lhsT, rhs=rhs, start=start,
                                 stop=stop, tile_position=pos,
                                 skip_group_check=True)

            def psum4():
                return [ps.tile([128, 512], F32, name=f"ps{nc.next_id()}",
                                tag="ps") for _ in range(4)]

            def evac_leaky(psum_view, bias_ap, dst, name, dve=False):
                if dve:
                    # DVE path: u = psum + bias, then in-place max(0.5u, u).
                    # Splitting evacs across ACT/DVE halves the per-stage
                    # PSUM-release latency (banks evacuate in parallel).
                    nc.vector.tensor_scalar_add(out=dst[:], in0=psum_view[:],
                                                scalar1=bias_ap)
                    nc.vector.scalar_tensor_tensor(
                        out=dst[:], in0=dst[:], scalar=SLOPE, in1=dst[:],
                        op0=ALU.mult, op1=ALU.max)
                    return
                if prelu:
                    # single-op: dst = Prelu(psum + bias), alpha=0.5 via AP
                    # (bit-exact; HW ignores float-imm alpha, honors AP alpha;
                    # CoreSim lacks Prelu, hence the 2-op fallback for sim)
                    nc.scalar.activation(dst[:], psum_view[:], AF.Prelu,
                                         bias=bias_ap, alpha=acol())
                    return
                u = tmp.tile(list(psum_view.shape), F32, name=f"u{name}", tag="u")
                nc.scalar.activation(u[:], psum_view[:], AF.Identity,
                                     bias=bias_ap)
                nc.vector.scalar_tensor_tensor(
                    out=dst[:], in0=u[:], scalar=SLOPE, in1=u[:],
                    op0=ALU.mult, op1=ALU.max)

            def l1(w):
                n = 465
                x0 = xin.tile([128, 4, n], F32, name=f"x0_{w}", tag="x0")
                nc.sync.dma_start(out=x0[:, :, :], in_=x_d[w, :, :, :])
                pt = psum4()
                for a in range(4):
                    for b in range(4):
                        mm(pt[a][32 * b:32 * b + 32, 0:n],
                           w_t[32 * a:32 * a + 32, C1IM:C1IM + 32],
                           x0[32 * a:32 * a + 32, b, :],
                           True, True, (32 * a, 32 * b))
                a1 = act.tile([128, 4, 31, 15], F32, name=f"a1_{w}", tag="a1",
                              bufs=3)
                for a in range(4):
                    evac_leaky(pt[a][:, 0:n], bcol(0), a1[:, a], f"1_{w}_{a}",
                               dve=(a % 2 == 1))
                # x+1-shifted replica on partitions 16..32 of each b-block.
                # Flat shift over the merged free dim: contiguous runs; the
                # polluted last column (x=14) is never read by L2 windows.
                a1f = a1.rearrange("p a h w -> p (a h w)")
                for b in range(4):
                    nc.sync.dma_start(
                        out=a1f[32 * b + 16:32 * b + 32, 0:1859],
                        in_=a1f[32 * b:32 * b + 16, 1:1860])
                S[w]["a1"] = a1

            def l2(w):
                _, ci, co, hi, wi, ho, wo = LAYERS[1]
                n = ho * wo
                a1 = S[w]["a1"]
                pt = psum4()
                for dy in range(2):
                    for b in range(4):
                        for a in range(4):
                            mm(pt[b][32 * a:32 * a + 32, 0:n],
                               w_t[32 * b:32 * b + 32, C2F[dy]:C2F[dy] + 32],
                               a1[32 * b:32 * b + 32, a, dy:dy + ho, 0:wo],
                               dy == 0, dy == 1, (32 * b, 32 * a))
                a2 = act.tile([128, 4, ho, wo], F32, name=f"a2_{w}", tag="a2")
                for b in range(4):
                    evac_leaky(pt[b][:, 0:n], bcol(1), a2[:, b], f"2_{w}_{b}",
                               dve=(b % 2 == 1))
                S[w]["a2"] = a2

            def l3(w, sb):
                _, ci, co, hi, wi, ho, wo = LAYERS[2]
                n = ho * wo
                a2 = S[w]["a2"]
                pt = psum4()
                for t in range(4):
                    dy, dx = t // 2, t % 2
                    for a in range(4):
                        for bb in range(2):
                            b = 2 * sb + bb
                            mm(pt[a][64 * bb:64 * bb + 64, 0:n],
                               w_t[32 * a:32 * a + ci, C3T[t]:C3T[t] + 64],
                               a2[32 * a:32 * a + ci, b,
                                  dy:dy + ho, dx:dx + wo],
                               t == 0, t == 3, (32 * a, 64 * bb))
                a3 = pads["a3A" if sb == 0 else "a3B"][w % 3]
                for a in range(4):
                    evac_leaky(pt[a][:, 0:n], bcol(2),
                               a3[:, a, 1:1 + ho, 1:1 + wo], f"3_{w}_{sb}_{a}",
                               dve=(a % 2 == 1))
                S[w][f"a3{sb}"] = a3

            def l4(w, sb):
                _, ci, co, hi, wi, ho, wo = LAYERS[3]
                n = ho * wo
                a3 = S[w][f"a3{sb}"]
                pt = [ps.tile([128, 512], F32, name=f"ps{nc.next_id()}",
                              tag="ps") for _ in range(2)]
                for t in range(4):
                    dy, dx = t // 2, t % 2
                    for a in range(4):
                        for bb in range(2):
                            mm(pt[bb][32 * a:32 * a + 32, 0:n],
                               w_t[64 * bb:64 * bb + 64, C4F[t]:C4F[t] + 32],
                               a3[64 * bb:64 * bb + 64, a,
                                  dy:dy + ho, dx:dx + wo],
                               t == 0, t == 3, (64 * bb, 32 * a))
                a4 = pads["a4"][w % 3]
                S[w]["a4"] = a4
                for bb in range(2):
                    b = 2 * sb + bb
                    evac_leaky(pt[bb][:, 0:n], bcol(3),
                               a4[:, b, 1:1 + ho, 1:1 + wo], f"4_{w}_{b}",
                               dve=(bb % 2 == 1))

            def l5(w):
                _, ci, co, hi, wi, ho, wo = LAYERS[4]
                n = ho * wo
                a4 = S[w]["a4"]
                pt = psum4()
                for t in range(4):
                    dy, dx = t // 2, t % 2
                    mw = 32 if t in (0, 3) else 16
                    for a in range(4):
                        for b in range(4):
                            mm(pt[a][32 * b:32 * b + mw, 0:n],
                               w_t[32 * a:32 * a + ci, C5T[t]:C5T[t] + mw],
                               a4[32 * a:32 * a + ci, b, dy:dy + ho, dx:dx + wo],
                               t == 0, t == 3, (32 * a, 32 * b))
                a5 = pads["a5"][w % 3]
                for a in range(4):
                    evac_leaky(pt[a][:, 0:n], bcol(4),
                               a5[:, a, 1:1 + ho, 1:1 + wo], f"5_{w}_{a}",
                               dve=(a % 2 == 1))
                a5f = a5.rearrange("p a h w -> p (a h w)")
                for b in range(4):
                    nc.sync.dma_start(
                        out=a5f[32 * b + 16:32 * b + 32, 0:2243],
                        in_=a5f[32 * b:32 * b + 16, 1:2244])
                S[w]["a5"] = a5

            def l6(w):
                _, ci, co, hi, wi, ho, wo = LAYERS[5]
                n = ho * wo
                a5 = S[w]["a5"]
                pt = psum4()
                for dy in range(2):
                    for b in range(4):
                        for a in range(4):
                            mm(pt[b][32 * a:32 * a + 32, 0:n],
                               w_t[32 * b:32 * b + 32, C6F[dy]:C6F[dy] + 32],
                               a5[32 * b:32 * b + 32, a, dy:dy + ho, 0:wo],
                               dy == 0, dy == 1, (32 * b, 32 * a))
                o6 = act.tile([128, 4, ho * wo], F32, name=f"a6_{w}", tag="a6",
                              bufs=2)
                for b in range(4):
                    nc.scalar.activation(o6[:, b, :], pt[b][:, 0:n],
                                         AF.Identity, bias=bcol(5))
                    nc.sync.dma_start(out=y_d[w, b, :, :, :],
                                      in_=o6[0:128:32, b, :])

            def emit(w, st):
                if st == 0:
                    l1(w)
                elif st == 1:
                    l2(w)
                elif st in (2, 3):
                    l3(w, st - 2)
                elif st in (4, 5):
                    l4(w, st - 4)
                elif st == 6:
                    l5(w)
                else:
                    l6(w)

            NST = 8
            sched = sorted(
                ((max(w - 2, 0) if s == 0 else w + s), s, -w, w)
                for w in range(waves) for s in range(NST))
            for _, s, _, w in sched:
                emit(w, s)

    nc.compile()
    return nc


_NC_CACHE = {}


def _get_nc():
    if "nc" not in _NC_CACHE:
        _NC_CACHE["nc"] = build()
    return _NC_CACHE["nc"]


def _post_process(Rx, M):
    import jax
    import jax.numpy as jnp

    cpu = jax.devices("cpu")[0]
    with jax.default_device(cpu):
        n = NSEN
        Rx = jnp.asarray(Rx)
        K = Rx[:, :n, :] + 1j * Rx[:, n:, :]
        g = jnp.einsum("bij,bik->bjk", jnp.conj(K), K)
        Rz = g + EPS * jnp.eye(n, dtype=g.dtype)

        w, v = jnp.linalg.eig(Rz)
        order = jnp.argsort(jnp.abs(w), axis=-1)[:, ::-1]
        Un = jnp.take_along_axis(v, order[:, None, :], axis=2)[:, :, M:]
        F = jnp.einsum("bik,bjk->bij", Un, jnp.conj(Un))
        ids = (jnp.arange(n)[None, :] - jnp.arange(n)[:, None] + n - 1).ravel()
        seg = lambda f: jax.ops.segment_sum(f.ravel(), ids, num_segments=2 * n - 1)
        coeff = jax.vmap(seg)(F)
        L = 2 * n - 2
        A = jnp.broadcast_to(jnp.eye(L, k=-1, dtype=coeff.dtype),
                             (coeff.shape[0], L, L))
        A = A.at[:, 0, :].set(-coeff[:, 1:] / coeff[:, :1])
        roots = jnp.linalg.eig(A)[0]
        inv_pi = 1.0 / np.pi
        DOA_all = jnp.arcsin(inv_pi * jnp.angle(roots))
        o2 = jnp.argsort(jnp.abs(jnp.abs(roots) - 1.0), axis=-1)
        roots_s = jnp.take_along_axis(roots, o2, axis=-1)
        sorted_angels = jnp.arcsin(inv_pi * jnp.angle(roots_s))
        inside = (jnp.abs(roots_s) - 1.0) < 0
        o3 = jnp.argsort((~inside).astype(jnp.int32), axis=-1, stable=True)[:, :M]
        roots_in = jnp.take_along_axis(roots_s, o3, axis=-1)
        DOA = jnp.arcsin(inv_pi * jnp.angle(roots_in))
        return (np.asarray(DOA), np.asarray(DOA_all), np.asarray(roots_s[-1]),
                np.asarray(Rz), np.asarray(sorted_angels))


def run_cnn_device(New_Rx_tau, params, trace=False):
    """Run the CNN on the 8 NeuronCores; returns Rx [B, 32, 16] float32."""
    nc = _get_nc()
    wblob = _weights_blob(params)
    x = np.asarray(New_Rx_tau, np.float32)
    in_maps = []
    for c in range(N_CORES):
        in_maps.append({"x": _pack_x(x[c * BPC:(c + 1) * BPC]), "w": wblob})
    res = run_bass_kernel_spmd(nc, in_maps, list(range(N_CORES)), trace=trace)
    _NC_CACHE["last_res"] = res
    return np.concatenate(
        [_unpack_y(res.results[c]["y"]) for c in range(N_CORES)], axis=0)


def kernel(New_Rx_tau, M, conv1_w, conv1_b, conv2_w, conv2_b, conv3_w, conv3_b,
           deconv2_w, deconv2_b, deconv3_w, deconv3_b, deconv4_w, deconv4_b):
    params = dict(conv1_w=np.asarray(conv1_w), conv1_b=np.asarray(conv1_b),
                  conv2_w=np.asarray(conv2_w), conv2_b=np.asarray(conv2_b),
                  conv3_w=np.asarray(conv3_w), conv3_b=np.asarray(conv3_b),
                  deconv2_w=np.asarray(deconv2_w), deconv2_b=np.asarray(deconv2_b),
                  deconv3_w=np.asarray(deconv3_w), deconv3_b=np.asarray(deconv3_b),
                  deconv4_w=np.asarray(deconv4_w), deconv4_b=np.asarray(deconv4_b))
    Rx = run_cnn_device(np.asarray(New_Rx_tau), params)
    return _post_process(Rx, int(M))


def golden_cnn(x, p):
    """Numpy float64 reference of the CNN (debugging aid)."""
    def lrelu(v):
        return np.where(v >= 0, v, SLOPE * v)

    def conv(v, w, b):
        co, ci, _, _ = w.shape
        B, _, h, ww = v.shape
        out = np.zeros((B, co, h - 1, ww - 1), v.dtype)
        for dy in range(2):
            for dx in range(2):
                out += np.einsum("bihw,oi->bohw",
                                 v[:, :, dy:dy + h - 1, dx:dx + ww - 1],
                                 w[:, :, dy, dx])
        return out + b[None, :, None, None]

    def deconv(v, w, b):
        ci, co, _, _ = w.shape
        B, _, h, ww = v.shape
        out = np.zeros((B, co, h + 1, ww + 1), v.dtype)
        for dy in range(2):
            for dx in range(2):
                out[:, :, 1 - dy:1 - dy + h, 1 - dx:1 - dx + ww] += np.einsum(
                    "bihw,io->bohw", v, w[:, :, 1 - dy, 1 - dx])
        return out + b[None, :, None, None]

    x = x.astype(np.float64)
    p64 = {k: v.astype(np.float64) for k, v in p.items()}
    v = lrelu(conv(x, p64["conv1_w"], p64["conv1_b"]))
    v = lrelu(conv(v, p64["conv2_w"], p64["conv2_b"]))
    v = lrelu(conv(v, p64["conv3_w"], p64["conv3_b"]))
    v = lrelu(deconv(v, p64["deconv2_w"], p64["deconv2_b"]))
    v = lrelu(deconv(v, p64["deconv3_w"], p64["deconv3_b"]))
    v = deconv(v, p64["deconv4_w"], p64["deconv4_b"])
    return v[:, 0]
